# revision 10
# baseline (speedup 1.0000x reference)
"""Graphormer attention head on 8 trn2 NeuronCores (row-parallel).

out = softmax(mask(q@k.T/8, adj)) @ v  with q/k/v = x@W+b, adj scattered
from edge_index.  Core c owns output rows [c*1024, (c+1)*1024).

Design (per core):
- Projections in fp16. No bias adds on device: per-q-row bias terms
  cancel in softmax; the per-k-row term c_j = (bq*scale)@k_j is an
  extra output column of the V projection, applied as the per-partition
  bias of the Exp activation.
- Scores: q,k split hi+lo in fp8e4; one DoubleRow matmul per [128,512]
  output computes the full (qh+ql)(kh+kl) product: dims 0-63 sit at
  partitions 0-63 with (hi,hi)/(lo,lo) pairing and are duplicated at
  partitions 64-127 with the q pairing swapped.  The K/Q projections
  use column-duplicated weights so the PSUM result is already
  replicated across both partition halves.
- Mask: additive offsets {-2 edge, -28 non-edge} in fp8, added into
  the scores PSUM by DoubleRow matmuls with (I,0)/(0,I) identity
  stationaries (a mask tile carries two adjacent jt tiles). The -2
  shift is a softmax-invariant that bounds exp outputs.
- Exp on Act engine (the ~66us floor): PSUM f32 -> fp16 w tiles,
  with the c_j bias fused.  No per-tile DVE work.
- attn@v in fp16: one matmul per tile accumulating [66,1024]; a ones
  column in V yields the softmax denominator; a final small matmul
  with I66 transposes and folds bv.
- DMA triggers cost ~600ns of queue time each, so transfers are
  batched (one per x^T segment, one per 4 mask tiles) and mask loads
  issue from the otherwise-idle GpSimd queue; projection work for
  x^T segment s+1 is emitted inside main-loop block s-1 so the PE
  stream never stalls the Act engine.
"""
import os
import sys

for _p in ("/opt/trn_rl_repo", "/root/.axon_site/_ro/trn_rl_repo"):
    if os.path.isdir(_p) and _p not in sys.path:
        sys.path.insert(0, _p)

import numpy as np
import ml_dtypes

import concourse.bass as bass
import concourse.bacc as bacc
import concourse.mybir as mybir
import concourse.tile as tile
from concourse.bass_utils import run_bass_kernel_spmd

N = 8192
DIN = 256
DQ = 64
NCORES = 8
NLOC = N // NCORES          # 1024 rows per core
JT = N // 128               # 64 column tiles of 128
NSEG = 8                    # x^T streamed in 8 segments of 1024 columns
M = DQ + 2                  # v cols + ones col (64) + c col (65)
SHIFT = -2.0                # global score shift (cancels in softmax)
MOFF = -28.0                # additive mask for non-edges
F32 = mybir.dt.float32
F16 = mybir.dt.float16
FP8 = mybir.dt.float8e4


def _emit(nc, tc, ctx):
    from concourse.mybir import AluOpType as AO, ActivationFunctionType as AF
    DR = mybir.MatmulPerfMode.DoubleRow

    xt3 = nc.dram_tensor("xt3", [128, NSEG * 2 * 1024], F16,
                         kind="ExternalInput")
    xtq3 = nc.dram_tensor("xtq3", [128, 2 * NLOC], F16, kind="ExternalInput")
    wq3 = nc.dram_tensor("wq3", [128, 2 * 128], F16, kind="ExternalInput")
    wk3 = nc.dram_tensor("wk3", [128, 2 * 128], F16, kind="ExternalInput")
    wv3 = nc.dram_tensor("wv3", [128, 2 * M], F16, kind="ExternalInput")
    mask3 = nc.dram_tensor("mask3", [128, JT * NLOC], FP8,
                           kind="ExternalInput")
    id2 = nc.dram_tensor("id2", [128, 512], FP8, kind="ExternalInput")
    i66 = nc.dram_tensor("i66", [M, M], F32, kind="ExternalInput")
    out = nc.dram_tensor("out", [NLOC, DQ], F32, kind="ExternalOutput")

    pers = ctx.enter_context(tc.tile_pool(name="pers", bufs=1))
    pm = ctx.enter_context(tc.tile_pool(name="pm", bufs=3))
    pw = ctx.enter_context(tc.tile_pool(name="pw", bufs=4))
    pfin = ctx.enter_context(tc.tile_pool(name="pfin", bufs=2))
    ps = ctx.enter_context(tc.tile_pool(name="ps", bufs=2, space="PSUM"))
    pacc = ctx.enter_context(tc.tile_pool(name="pacc", bufs=1, space="PSUM"))
    pp = ctx.enter_context(tc.tile_pool(name="pp", bufs=2, space="PSUM"))

    # ---- persistent SBUF ----
    xt_sb = pers.tile([128, NSEG, 2, 1024], F16, tag="xt")
    xtq_sb = pers.tile([128, 2, NLOC], F16, tag="xtq")
    wq_sb = pers.tile([128, 2, 128], F16, tag="wq")
    wk_sb = pers.tile([128, 2, 128], F16, tag="wk")
    wv_sb = pers.tile([128, 2, M], F16, tag="wv")
    id2_sb = pers.tile([128, 512], FP8, tag="id2")
    i66_sb = pers.tile([M, M], F32, tag="i66")
    kk_sb = pers.tile([128, 2, N], FP8, tag="kk")      # (hi,lo), both halves
    q3_sb = pers.tile([128, 2, NLOC], FP8, tag="q3")   # (hi,lo)/(lo,hi)
    vh_sb = pers.tile([128, JT * M], F16, tag="vh")    # v blocks [128,66]
    c_sb = pers.tile([128, JT], F32, tag="c")
    accT_sb = pers.tile([M, NLOC], F32, tag="accT")
    o_all = pers.tile([128, (NLOC // 128) * DQ], F32, tag="oall")
    wu_sb = pers.tile([128, 8], F16, tag="wu")

    idv = id2_sb[:].rearrange("p (a i m) -> p a i m", a=2, i=2)
    vh3 = vh_sb[:].rearrange("p (b e) -> p b e", e=M)

    def xt_dma_early(s):
        nc.sync.dma_start(xt_sb[:, s, :, :].rearrange("p c n -> p (c n)"),
                          xt3[:, s * 2048:(s + 1) * 2048])
    xt_dma_early(0)
    nc.sync.dma_start(wq_sb[:].rearrange("p c j -> p (c j)"), wq3[:])
    nc.sync.dma_start(wk_sb[:].rearrange("p c j -> p (c j)"), wk3[:])
    nc.sync.dma_start(wv_sb[:].rearrange("p c j -> p (c j)"), wv3[:])
    nc.sync.dma_start(id2_sb[:], id2[:])
    nc.sync.dma_start(i66_sb[:], i66[:])
    nc.sync.dma_start(xtq_sb[:].rearrange("p c n -> p (c n)"), xtq3[:])

    # warm the Exp table before the main loop needs it
    nc.vector.memset(wu_sb[:], 0.0)
    nc.scalar.activation(wu_sb[:], wu_sb[:], AF.Exp)

    # ---- Q projection -> q3 (weights column-duplicated: PSUM rows
    # 64-127 replicate rows 0-63, so the swapped fp8 pairing needs no
    # partition-crossing copies) ----
    for h in range(2):
        hs = slice(h * 512, (h + 1) * 512)
        t = pp.tile([128, 512], F32, tag="pp", name=f"q{h}")
        nc.tensor.matmul(t[:], wq_sb[:, 0, :], xtq_sb[:, 0, hs],
                         start=True, stop=False)
        nc.tensor.matmul(t[:], wq_sb[:, 1, :], xtq_sb[:, 1, hs],
                         start=False, stop=True)
        nc.vector.tensor_scalar_add(q3_sb[0:64, 0, hs], t[0:64, :], 0.0)
        nc.vector.scalar_tensor_tensor(q3_sb[0:64, 1, hs], t[0:64, :], 1.0,
                                       q3_sb[0:64, 0, hs],
                                       AO.mult, AO.subtract)
        nc.vector.tensor_scalar_add(q3_sb[64:128, 1, hs], t[64:128, :], 0.0)
        nc.vector.scalar_tensor_tensor(q3_sb[64:128, 0, hs], t[64:128, :],
                                       1.0, q3_sb[64:128, 1, hs],
                                       AO.mult, AO.subtract)

    def xt_dma(s):
        nc.sync.dma_start(xt_sb[:, s, :, :].rearrange("p c n -> p (c n)"),
                          xt3[:, s * 2048:(s + 1) * 2048])

    def prep_k(s):
        # K: two 512-col halves; wk column-duplicated -> [128,512] PSUM
        for hh in range(2):
            cols = slice(s * 1024 + hh * 512, s * 1024 + (hh + 1) * 512)
            xs = slice(hh * 512, (hh + 1) * 512)
            t = pp.tile([128, 512], F32, tag="pp", name=f"k{s}_{hh}")
            nc.tensor.matmul(t[:], wk_sb[:, 0, :], xt_sb[:, s, 0, xs],
                             start=True, stop=False)
            nc.tensor.matmul(t[:], wk_sb[:, 1, :], xt_sb[:, s, 1, xs],
                             start=False, stop=True)
            nc.vector.tensor_scalar_add(kk_sb[:, 0, cols], t[:], 0.0)
            nc.vector.scalar_tensor_tensor(kk_sb[:, 1, cols], t[:], 1.0,
                                           kk_sb[:, 0, cols],
                                           AO.mult, AO.subtract)

    def prep_v(s, g2):
        # V: 4 blocks (+ c column extraction)
        if True:
            b0 = 8 * s + 4 * g2
            t = pp.tile([128, 4 * M], F32, tag="pp", name=f"v{s}_{g2}")
            for b in range(4):
                xs = slice((4 * g2 + b) * 128, (4 * g2 + b + 1) * 128)
                o = t[:, b * M:(b + 1) * M]
                nc.tensor.matmul(o, xt_sb[:, s, 0, xs], wv_sb[:, 0, :],
                                 start=True, stop=False)
                nc.tensor.matmul(o, xt_sb[:, s, 1, xs], wv_sb[:, 1, :],
                                 start=False, stop=True)
            t3 = t[:].rearrange("p (b e) -> p b e", e=M)
            nc.vector.tensor_scalar_add(vh_sb[:, b0 * M:(b0 + 4) * M], t[:],
                                        0.0)
            nc.vector.tensor_scalar_add(c_sb[:, b0:b0 + 4], t3[:, :, M - 1],
                                        0.0)
            nc.vector.memset(vh3[:, b0:b0 + 4, DQ:DQ + 1], 1.0)

    def mask_dma(q4):
        m_t = pm.tile([128, 4096], FP8, tag="m", name=f"m{q4}")
        nc.gpsimd.dma_start(m_t[:], mask3[:, q4 * 4096:(q4 + 1) * 4096])
        return m_t[:].rearrange("p (t i n) -> p t i n", t=2, i=2)

    mview = [None] * (JT // 4)
    mview[0] = mask_dma(0)
    mview[1] = mask_dma(1)
    prep_k(0)
    prep_v(0, 0)
    prep_v(0, 1)

    # ---- main loop over 64 k-tiles of 128 rows ----
    acc = pacc.tile([M, NLOC], F32, tag="acc")
    wts = [None] * JT

    def _emit_av(j):
        for h in range(2):
            hs = slice(h * 512, (h + 1) * 512)
            nc.tensor.matmul(acc[:, hs], vh3[:, j], wts[j][:, hs],
                             start=(j == 0), stop=(j == JT - 1))

    DVE_MASK = {1, 3, 5}
    for jt in range(JT):
        b = jt // 8
        if b + 1 < NSEG:
            if jt % 8 == 2:
                xt_dma(b + 1)
                prep_k(b + 1)
            elif jt % 8 == 4:
                prep_v(b + 1, 0)
            elif jt % 8 == 6:
                prep_v(b + 1, 1)
        if jt % 4 == 0 and jt // 4 + 2 < JT // 4:
            mview[jt // 4 + 2] = mask_dma(jt // 4 + 2)
        mv = mview[jt // 4]
        tsel = (jt % 4) // 2
        dve_mask = (jt % 8) in DVE_MASK
        s_t = ps.tile([128, NLOC], F32, tag="s", name=f"s{jt}")
        kts = kk_sb[:, :, jt * 128:(jt + 1) * 128]
        for h in range(2):
            hs = slice(h * 512, (h + 1) * 512)
            nc.tensor.matmul(s_t[:, hs], kts, q3_sb[:, :, hs],
                             start=True, stop=dve_mask, perf_mode=DR)
        if dve_mask:
            nc.vector.scalar_tensor_tensor(
                s_t[:], s_t[:], 1.0,
                mv[:, tsel, jt % 2, :], AO.mult, AO.add)
        else:
            for h in range(2):
                hs = slice(h * 512, (h + 1) * 512)
                nc.tensor.matmul(s_t[:, hs], idv[:, jt % 2],
                                 mv[:, tsel, :, hs],
                                 start=False, stop=True, perf_mode=DR)
        w_t = pw.tile([128, NLOC], F16, tag="w", name=f"w{jt}")
        wts[jt] = w_t
        nc.scalar.activation(w_t[:], s_t[:], AF.Exp, bias=c_sb[:, jt:jt + 1])
        if jt >= 1:
            _emit_av(jt - 1)
    _emit_av(JT - 1)

    # ---- finish: transpose via matmul with I66 (adds bv*Z), divide ----
    nc.vector.tensor_scalar_add(accT_sb[:, 0:512], acc[:, 0:512], 0.0)
    nc.vector.tensor_scalar_add(accT_sb[:, 512:1024], acc[:, 512:1024], 0.0)
    for it in range(NLOC // 128):
        po = pp.tile([128, M], F32, tag="pp", name=f"po{it}")
        nc.tensor.matmul(po[:], accT_sb[:, it * 128:(it + 1) * 128],
                         i66_sb[:], start=True, stop=True)
        rz = pfin.tile([128, 1], F32, tag="rz")
        nc.vector.reciprocal(rz[:], po[:, DQ:DQ + 1])
        nc.vector.tensor_scalar_mul(o_all[:, it * DQ:(it + 1) * DQ],
                                    po[:, 0:DQ], rz[:])
    nc.sync.dma_start(out[:].rearrange("(i p) c -> p i c", p=128),
                      o_all[:].rearrange("p (i c) -> p i c", c=DQ))


_CACHE = {}


def _program():
    if "nc" not in _CACHE:
        import contextlib
        nc = bacc.Bacc("TRN2", target_bir_lowering=False, debug=False,
                       num_devices=NCORES)
        with tile.TileContext(nc) as tc:
            with contextlib.ExitStack() as ctx:
                _emit(nc, tc, ctx)
        nc.compile()
        _CACHE["nc"] = nc
    return _CACHE["nc"]


def _chunk2(w):
    """[256, width] -> [128, 2*width] with w3[p, c*width+j] = w[c*128+p, j]."""
    width = w.shape[1]
    return np.ascontiguousarray(
        w.reshape(2, 128, width).transpose(1, 0, 2).reshape(128, 2 * width))


def kernel(**inputs):
    x = np.asarray(inputs["x"], dtype=np.float32)
    ei = np.asarray(inputs["edge_index"])
    Wq = np.asarray(inputs["Wq"], dtype=np.float32)
    bq = np.asarray(inputs["bq"], dtype=np.float32)
    Wk = np.asarray(inputs["Wk"], dtype=np.float32)
    Wv = np.asarray(inputs["Wv"], dtype=np.float32)
    bv = np.asarray(inputs["bv"], dtype=np.float32)
    # bk's score contribution is constant per q row: cancels in softmax

    FP8NP = ml_dtypes.float8_e4m3
    scale = np.float32(1.0 / np.sqrt(np.float32(DQ)))
    xT16 = x.T.astype(np.float16)                       # (256, 8192)
    wq_s = (Wq * scale).astype(np.float16)
    wk16 = Wk.astype(np.float16)
    wq3 = _chunk2(np.concatenate([wq_s, wq_s], axis=1))
    wk3 = _chunk2(np.concatenate([wk16, wk16], axis=1))
    w_c = Wk @ (bq * scale)                             # c_j = (bq*s).k_j
    wv_aug = np.zeros((DIN, M), np.float32)
    wv_aug[:, :DQ] = Wv
    wv_aug[:, DQ + 1] = w_c
    wv3 = _chunk2(wv_aug.astype(np.float16))
    xt3 = np.ascontiguousarray(
        xT16.reshape(2, 128, NSEG, 1024).transpose(1, 2, 0, 3)
        .reshape(128, NSEG * 2 * 1024))
    i66 = np.zeros((M, M), np.float32)
    i66[np.arange(DQ), np.arange(DQ)] = 1.0
    i66[DQ, :DQ] = bv
    i66[DQ, DQ] = 1.0
    id2 = np.zeros((128, 2, 2, 128), np.float32)
    r = np.arange(128)
    id2[r, 0, 0, r] = 1.0
    id2[r, 1, 1, r] = 1.0
    id2 = np.ascontiguousarray(id2.astype(FP8NP).reshape(128, 512))

    adj = np.zeros((N, N), dtype=np.bool_)
    adj[ei[0], ei[1]] = True

    in_maps = []
    for c in range(NCORES):
        rows = slice(c * NLOC, (c + 1) * NLOC)
        moff = np.where(adj[rows].T, np.float32(SHIFT), np.float32(MOFF))
        m3 = np.ascontiguousarray(
            moff.reshape(JT // 2, 2, 128, NLOC).transpose(2, 0, 1, 3)
            .astype(FP8NP).reshape(128, JT * NLOC))
        in_maps.append({
            "xt3": xt3,
            "xtq3": _chunk2(np.ascontiguousarray(xT16[:, rows])),
            "wq3": wq3, "wk3": wk3, "wv3": wv3,
            "mask3": m3, "id2": id2, "i66": i66,
        })

    global _last_in_maps
    _last_in_maps = in_maps
    nc = _program()
    res = run_bass_kernel_spmd(nc, in_maps, core_ids=list(range(NCORES)))
    out = np.concatenate([res.results[c]["out"] for c in range(NCORES)],
                         axis=0)
    return out.astype(np.float32)


_last_in_maps = None


# revision 14
# speedup vs baseline: 1.1510x; 1.1510x over previous
"""Graphormer attention head on 8 trn2 NeuronCores (row-parallel).

out = softmax(mask(q@k.T/8, adj)) @ v  with q/k/v = x@W+b, adj scattered
from edge_index.  Core c owns output rows [c*1024, (c+1)*1024).

Design (per core):
- Projections in fp16. No bias adds on device: per-q-row bias terms
  cancel in softmax; the per-k-row term c_j = (bq*scale)@k_j is an
  extra output column of the V projection, applied as the per-partition
  bias of the Exp activation.
- Scores: q,k split hi+lo in fp8e4; one DoubleRow matmul per [128,512]
  output computes the full (qh+ql)(kh+kl) product: dims 0-63 sit at
  partitions 0-63 with (hi,hi)/(lo,lo) pairing and are duplicated at
  partitions 64-127 with the q pairing swapped.  The K/Q projections
  use column-duplicated weights so the PSUM result is already
  replicated across both partition halves.
- Mask: additive offsets {-2 edge, -28 non-edge} in fp8, added into
  the scores PSUM by DoubleRow matmuls with (I,0)/(0,I) identity
  stationaries (a mask tile carries two adjacent jt tiles). The -2
  shift is a softmax-invariant that bounds exp outputs.
- Exp on Act engine (the ~66us floor): PSUM f32 -> fp16 w tiles,
  with the c_j bias fused.  No per-tile DVE work.
- attn@v in fp16: one matmul per tile accumulating [66,1024]; a ones
  column in V yields the softmax denominator; a final small matmul
  with I66 transposes and folds bv.
- DMA triggers cost ~600ns of queue time each, so transfers are
  batched (one per x^T segment, one per 4 mask tiles) and mask loads
  issue from the otherwise-idle GpSimd queue; projection work for
  x^T segment s+1 is emitted inside main-loop block s-1 so the PE
  stream never stalls the Act engine.
"""
import os
import sys

for _p in ("/opt/trn_rl_repo", "/root/.axon_site/_ro/trn_rl_repo"):
    if os.path.isdir(_p) and _p not in sys.path:
        sys.path.insert(0, _p)

import numpy as np
import ml_dtypes

import concourse.bass as bass
import concourse.bacc as bacc
import concourse.mybir as mybir
import concourse.tile as tile
from concourse.bass_utils import run_bass_kernel_spmd

N = 8192
DIN = 256
DQ = 64
NCORES = 8
NLOC = N // NCORES          # 1024 rows per core
JT = N // 128               # 64 column tiles of 128
NSEG = 8                    # x^T streamed in 8 segments of 1024 columns
M = DQ + 2                  # v cols + ones col (64) + c col (65)
SHIFT = -2.0                # global score shift (cancels in softmax)
MOFF = -28.0                # additive mask for non-edges
F32 = mybir.dt.float32
F16 = mybir.dt.float16
FP8 = mybir.dt.float8e4


def _emit(nc, tc, ctx):
    from concourse.mybir import AluOpType as AO, ActivationFunctionType as AF
    DR = mybir.MatmulPerfMode.DoubleRow

    xt3 = nc.dram_tensor("xt3", [128, NSEG * 2 * 1024], F16,
                         kind="ExternalInput")
    xtq3 = nc.dram_tensor("xtq3", [128, 2 * NLOC], F16, kind="ExternalInput")
    wq3 = nc.dram_tensor("wq3", [128, 2 * 128], F16, kind="ExternalInput")
    wk3 = nc.dram_tensor("wk3", [128, 2 * 128], F16, kind="ExternalInput")
    wv3 = nc.dram_tensor("wv3", [128, 2 * M], F16, kind="ExternalInput")
    mask3 = nc.dram_tensor("mask3", [128, JT * NLOC], FP8,
                           kind="ExternalInput")
    id2 = nc.dram_tensor("id2", [128, 512], FP8, kind="ExternalInput")
    i66 = nc.dram_tensor("i66", [M, M], F16, kind="ExternalInput")
    out = nc.dram_tensor("out", [NLOC, DQ], F32, kind="ExternalOutput")

    pers = ctx.enter_context(tc.tile_pool(name="pers", bufs=1))
    pm = ctx.enter_context(tc.tile_pool(name="pm", bufs=3))
    pw = ctx.enter_context(tc.tile_pool(name="pw", bufs=4))
    pfin = ctx.enter_context(tc.tile_pool(name="pfin", bufs=2))
    ps = ctx.enter_context(tc.tile_pool(name="ps", bufs=2, space="PSUM"))
    pacc = ctx.enter_context(tc.tile_pool(name="pacc", bufs=1, space="PSUM"))
    pp = ctx.enter_context(tc.tile_pool(name="pp", bufs=2, space="PSUM"))

    # ---- persistent SBUF ----
    xt_sb = pers.tile([128, NSEG, 2, 1024], F16, tag="xt")
    xtq_sb = pers.tile([128, 2, 2, 512], F16, tag="xtq")
    wq_sb = pers.tile([128, 2, 128], F16, tag="wq")
    wk_sb = pers.tile([128, 2, 128], F16, tag="wk")
    wv_sb = pers.tile([128, 2, M], F16, tag="wv")
    id2_sb = pers.tile([128, 512], FP8, tag="id2")
    i66_sb = pers.tile([M, M], F16, tag="i66")
    kk_sb = pers.tile([128, 2, N], FP8, tag="kk")      # (hi,lo), both halves
    q3_sb = pers.tile([128, 2, NLOC], FP8, tag="q3")   # (hi,lo)/(lo,hi)
    vh_sb = pers.tile([128, JT * M], F16, tag="vh")    # v blocks [128,66]
    c_sb = pers.tile([128, JT], F32, tag="c")
    accT_sb = pers.tile([M, NLOC], F16, tag="accT")
    o_all = pers.tile([128, (NLOC // 128) * DQ], F32, tag="oall")
    wu_sb = pers.tile([128, 8], F16, tag="wu")

    idv = id2_sb[:].rearrange("p (a i m) -> p a i m", a=2, i=2)
    vh3 = vh_sb[:].rearrange("p (b e) -> p b e", e=M)

    # earliest-needed first; sync queue serializes triggers at ~600ns
    nc.sync.dma_start(xtq_sb[:, 0, :, :].rearrange("p c n -> p (c n)"),
                      xtq3[:, 0:1024])
    nc.sync.dma_start(wq_sb[:].rearrange("p c j -> p (c j)"), wq3[:])
    nc.sync.dma_start(xtq_sb[:, 1, :, :].rearrange("p c n -> p (c n)"),
                      xtq3[:, 1024:2048])
    nc.sync.dma_start(wk_sb[:].rearrange("p c j -> p (c j)"), wk3[:])
    nc.sync.dma_start(xt_sb[:, 0, :, :].rearrange("p c n -> p (c n)"),
                      xt3[:, 0:2048])
    nc.sync.dma_start(xt_sb[:, 1, :, :].rearrange("p c n -> p (c n)"),
                      xt3[:, 2048:4096])
    nc.gpsimd.dma_start(wv_sb[:].rearrange("p c j -> p (c j)"), wv3[:])
    nc.gpsimd.dma_start(id2_sb[:], id2[:])
    nc.gpsimd.dma_start(i66_sb[:], i66[:])

    # warm the Exp table before the main loop needs it
    nc.vector.memset(wu_sb[:], 0.0)
    nc.scalar.activation(wu_sb[:], wu_sb[:], AF.Exp)

    # ---- Q projection -> q3 (weights column-duplicated: PSUM rows
    # 64-127 replicate rows 0-63, so the swapped fp8 pairing needs no
    # partition-crossing copies) ----
    for h in range(2):
        hs = slice(h * 512, (h + 1) * 512)
        t = pp.tile([128, 512], F32, tag="pp", name=f"q{h}")
        nc.tensor.matmul(t[:], wq_sb[:, 0, :], xtq_sb[:, h, 0, :],
                         start=True, stop=False)
        nc.tensor.matmul(t[:], wq_sb[:, 1, :], xtq_sb[:, h, 1, :],
                         start=False, stop=True)
        nc.scalar.copy(q3_sb[0:64, 0, hs], t[0:64, :])
        nc.vector.scalar_tensor_tensor(q3_sb[0:64, 1, hs], t[0:64, :], 1.0,
                                       q3_sb[0:64, 0, hs],
                                       AO.mult, AO.subtract)
        nc.scalar.copy(q3_sb[64:128, 1, hs], t[64:128, :])
        nc.vector.scalar_tensor_tensor(q3_sb[64:128, 0, hs], t[64:128, :],
                                       1.0, q3_sb[64:128, 1, hs],
                                       AO.mult, AO.subtract)

    def xt_dma(s):
        nc.sync.dma_start(xt_sb[:, s, :, :].rearrange("p c n -> p (c n)"),
                          xt3[:, s * 2048:(s + 1) * 2048])

    def prep_k(s, early=False):
        # K: two 512-col halves; wk column-duplicated -> [128,512] PSUM
        for hh in range(2):
            cols = slice(s * 1024 + hh * 512, s * 1024 + (hh + 1) * 512)
            xs = slice(hh * 512, (hh + 1) * 512)
            t = pp.tile([128, 512], F32, tag="pp", name=f"k{s}_{hh}")
            nc.tensor.matmul(t[:], wk_sb[:, 0, :], xt_sb[:, s, 0, xs],
                             start=True, stop=False)
            nc.tensor.matmul(t[:], wk_sb[:, 1, :], xt_sb[:, s, 1, xs],
                             start=False, stop=True)
            if early:
                nc.scalar.copy(kk_sb[:, 0, cols], t[:])
            else:
                nc.vector.tensor_scalar_add(kk_sb[:, 0, cols], t[:], 0.0)
            nc.vector.scalar_tensor_tensor(kk_sb[:, 1, cols], t[:], 1.0,
                                           kk_sb[:, 0, cols],
                                           AO.mult, AO.subtract)

    def prep_v(s, g2, early=False):
        # V: 4 blocks (+ c column extraction)
        if True:
            b0 = 8 * s + 4 * g2
            t = pp.tile([128, 4 * M], F32, tag="pp", name=f"v{s}_{g2}")
            for b in range(4):
                xs = slice((4 * g2 + b) * 128, (4 * g2 + b + 1) * 128)
                o = t[:, b * M:(b + 1) * M]
                nc.tensor.matmul(o, xt_sb[:, s, 0, xs], wv_sb[:, 0, :],
                                 start=True, stop=False)
                nc.tensor.matmul(o, xt_sb[:, s, 1, xs], wv_sb[:, 1, :],
                                 start=False, stop=True)
            t3 = t[:].rearrange("p (b e) -> p b e", e=M)
            if early:
                nc.scalar.copy(vh_sb[:, b0 * M:(b0 + 4) * M], t[:])
            else:
                nc.vector.tensor_scalar_add(vh_sb[:, b0 * M:(b0 + 4) * M],
                                            t[:], 0.0)
            nc.vector.tensor_scalar_add(c_sb[:, b0:b0 + 4], t3[:, :, M - 1],
                                        0.0)
            nc.vector.memset(vh3[:, b0:b0 + 4, DQ:DQ + 1], 1.0)

    def mask_dma(q4):
        m_t = pm.tile([128, 4096], FP8, tag="m", name=f"m{q4}")
        nc.gpsimd.dma_start(m_t[:], mask3[:, q4 * 4096:(q4 + 1) * 4096])
        return m_t[:].rearrange("p (t i n) -> p t i n", t=2, i=2)

    mview = [None] * (JT // 4)
    mview[0] = mask_dma(0)
    mview[1] = mask_dma(1)
    prep_k(0, early=True)
    prep_v(0, 0, early=True)
    prep_v(0, 1, early=True)

    # ---- main loop over 64 k-tiles of 128 rows ----
    acc = pacc.tile([M, NLOC], F32, tag="acc")
    wts = [None] * JT

    def _emit_av(j):
        for h in range(2):
            hs = slice(h * 512, (h + 1) * 512)
            nc.tensor.matmul(acc[:, hs], vh3[:, j], wts[j][:, hs],
                             start=(j == 0), stop=(j == JT - 1))

    DVE_MASK = set()
    for jt in range(JT):
        b = jt // 8
        if jt % 8 == 2 and b + 2 < NSEG:
            xt_dma(b + 2)
        if b + 1 < NSEG:
            if jt % 8 == 2:
                prep_k(b + 1)
            elif jt % 8 == 4:
                prep_v(b + 1, 0)
            elif jt % 8 == 6:
                prep_v(b + 1, 1)
        if jt % 4 == 0 and jt // 4 + 2 < JT // 4:
            mview[jt // 4 + 2] = mask_dma(jt // 4 + 2)
        mv = mview[jt // 4]
        tsel = (jt % 4) // 2
        dve_mask = (jt % 8) in DVE_MASK
        s_t = ps.tile([128, NLOC], F32, tag="s", name=f"s{jt}")
        kts = kk_sb[:, :, jt * 128:(jt + 1) * 128]
        for h in range(2):
            hs = slice(h * 512, (h + 1) * 512)
            nc.tensor.matmul(s_t[:, hs], kts, q3_sb[:, :, hs],
                             start=True, stop=dve_mask, perf_mode=DR)
        if dve_mask:
            nc.vector.scalar_tensor_tensor(
                s_t[:], s_t[:], 1.0,
                mv[:, tsel, jt % 2, :], AO.mult, AO.add)
        else:
            for h in range(2):
                hs = slice(h * 512, (h + 1) * 512)
                nc.tensor.matmul(s_t[:, hs], idv[:, jt % 2],
                                 mv[:, tsel, :, hs],
                                 start=False, stop=True, perf_mode=DR)
        w_t = pw.tile([128, NLOC], F16, tag="w", name=f"w{jt}")
        wts[jt] = w_t
        nc.scalar.activation(w_t[:], s_t[:], AF.Exp, bias=c_sb[:, jt:jt + 1])
        if jt >= 1:
            _emit_av(jt - 1)
    _emit_av(JT - 1)

    # ---- finish: transpose via matmul with I66 (adds bv*Z), divide ----
    nc.vector.tensor_scalar_add(accT_sb[:, 0:512], acc[:, 0:512], 0.0)
    nc.vector.tensor_scalar_add(accT_sb[:, 512:1024], acc[:, 512:1024], 0.0)
    for it in range(NLOC // 128):
        po = pp.tile([128, M], F32, tag="pp", name=f"po{it}")
        nc.tensor.matmul(po[:], accT_sb[:, it * 128:(it + 1) * 128],
                         i66_sb[:], start=True, stop=True)
        rz = pfin.tile([128, 1], F32, tag="rz")
        nc.vector.reciprocal(rz[:], po[:, DQ:DQ + 1])
        nc.vector.tensor_scalar_mul(o_all[:, it * DQ:(it + 1) * DQ],
                                    po[:, 0:DQ], rz[:])
    nc.sync.dma_start(out[:].rearrange("(i p) c -> p i c", p=128),
                      o_all[:].rearrange("p (i c) -> p i c", c=DQ))


_CACHE = {}


def _program():
    if "nc" not in _CACHE:
        import contextlib
        nc = bacc.Bacc("TRN2", target_bir_lowering=False, debug=False,
                       num_devices=NCORES)
        with tile.TileContext(nc) as tc:
            with contextlib.ExitStack() as ctx:
                _emit(nc, tc, ctx)
        nc.compile()
        _CACHE["nc"] = nc
    return _CACHE["nc"]


def _chunk2(w):
    """[256, width] -> [128, 2*width] with w3[p, c*width+j] = w[c*128+p, j]."""
    width = w.shape[1]
    return np.ascontiguousarray(
        w.reshape(2, 128, width).transpose(1, 0, 2).reshape(128, 2 * width))


def kernel(**inputs):
    x = np.asarray(inputs["x"], dtype=np.float32)
    ei = np.asarray(inputs["edge_index"])
    Wq = np.asarray(inputs["Wq"], dtype=np.float32)
    bq = np.asarray(inputs["bq"], dtype=np.float32)
    Wk = np.asarray(inputs["Wk"], dtype=np.float32)
    Wv = np.asarray(inputs["Wv"], dtype=np.float32)
    bv = np.asarray(inputs["bv"], dtype=np.float32)
    # bk's score contribution is constant per q row: cancels in softmax

    FP8NP = ml_dtypes.float8_e4m3
    scale = np.float32(1.0 / np.sqrt(np.float32(DQ)))
    xT16 = x.T.astype(np.float16)                       # (256, 8192)
    wq_s = (Wq * scale).astype(np.float16)
    wk16 = Wk.astype(np.float16)
    wq3 = _chunk2(np.concatenate([wq_s, wq_s], axis=1))
    wk3 = _chunk2(np.concatenate([wk16, wk16], axis=1))
    w_c = Wk @ (bq * scale)                             # c_j = (bq*s).k_j
    wv_aug = np.zeros((DIN, M), np.float32)
    wv_aug[:, :DQ] = Wv
    wv_aug[:, DQ + 1] = w_c
    wv3 = _chunk2(wv_aug.astype(np.float16))
    xt3 = np.ascontiguousarray(
        xT16.reshape(2, 128, NSEG, 1024).transpose(1, 2, 0, 3)
        .reshape(128, NSEG * 2 * 1024))
    i66 = np.zeros((M, M), np.float32)
    i66[np.arange(DQ), np.arange(DQ)] = 1.0
    i66[DQ, :DQ] = bv
    i66[DQ, DQ] = 1.0
    i66 = i66.astype(np.float16)
    id2 = np.zeros((128, 2, 2, 128), np.float32)
    r = np.arange(128)
    id2[r, 0, 0, r] = 1.0
    id2[r, 1, 1, r] = 1.0
    id2 = np.ascontiguousarray(id2.astype(FP8NP).reshape(128, 512))

    adj = np.zeros((N, N), dtype=np.bool_)
    adj[ei[0], ei[1]] = True

    in_maps = []
    for c in range(NCORES):
        rows = slice(c * NLOC, (c + 1) * NLOC)
        moff = np.where(adj[rows].T, np.float32(SHIFT), np.float32(MOFF))
        m3 = np.ascontiguousarray(
            moff.reshape(JT // 2, 2, 128, NLOC).transpose(2, 0, 1, 3)
            .astype(FP8NP).reshape(128, JT * NLOC))
        in_maps.append({
            "xt3": xt3,
            "xtq3": np.ascontiguousarray(
                xT16[:, rows].reshape(2, 128, 2, 512).transpose(1, 2, 0, 3)
                .reshape(128, 2 * NLOC)),
            "wq3": wq3, "wk3": wk3, "wv3": wv3,
            "mask3": m3, "id2": id2, "i66": i66,
        })

    global _last_in_maps
    _last_in_maps = in_maps
    nc = _program()
    res = run_bass_kernel_spmd(nc, in_maps, core_ids=list(range(NCORES)))
    out = np.concatenate([res.results[c]["out"] for c in range(NCORES)],
                         axis=0)
    return out.astype(np.float32)


_last_in_maps = None
